# revision 9
# baseline (speedup 1.0000x reference)
"""AdaFaceV3 head: out = S * cos_m where cos_m is clip(cos) with an
angular/additive margin applied only at (i, label[i]).

Math used here: for non-label entries cos(arccos(x)) == x and the theta
clip provably never binds (cosine already clipped to +-(1-1e-3)), so the
bulk of the output is just S * clip(emb @ kn, +-(1-eps)) -- a matmul with
per-column scaling 1/clip(||kcol||, 1e-5). Only the B label entries need
the arccos/cos margin chain, computed on-device via arctan/sin LUTs.

Sharding: kernel columns (class dim C) split across 8 cores; each core
computes its [B, C/8] logit slice. Fix-up values (one per batch row) are
computed redundantly on every core; host scatters core 0's copy during
unsharding.
"""

import math

import numpy as np

import concourse.bass as bass
import concourse.mybir as mybir
import concourse.tile as tile
from concourse import bacc
from concourse.bass_utils import run_bass_kernel_spmd

B = 1024
D = 512
C = 51332
NCORES = 8
CS = 6528          # per-core padded column count (51 * 128)
CPAD = CS * NCORES

EPS = 1e-3
M_MARGIN = 0.5
H = 0.333
S = 64.0
HEAD_B = 0.5
BSTD = 100.0

F32 = mybir.dt.float32
F32R = mybir.dt.float32r
AF = mybir.ActivationFunctionType
ALU = mybir.AluOpType

_nc_cache = {}


def _c_tiles():
    """(c0, cn) tiles of the per-core column range, N<=512 (one PSUM bank)."""
    tiles = []
    c0 = 0
    while c0 < CS:
        cn = min(512, CS - c0)
        tiles.append((c0, cn))
        c0 += cn
    return tiles


def build_nc():
    nc = bacc.Bacc("TRN2", target_bir_lowering=False, debug=False,
                   num_devices=NCORES)

    ksh = nc.dram_tensor("ksh", [D, CS], F32, kind="ExternalInput")
    embT = nc.dram_tensor("embT", [D, B], F32, kind="ExternalInput")
    emb = nc.dram_tensor("emb", [B, D], F32, kind="ExternalInput")
    klabT = nc.dram_tensor("klabT", [B, D], F32, kind="ExternalInput")
    norms = nc.dram_tensor("norms", [B, 1], F32, kind="ExternalInput")
    out = nc.dram_tensor("out", [B, CS], F32, kind="ExternalOutput")
    fixv = nc.dram_tensor("fixv", [B, 1], F32, kind="ExternalOutput")

    ND = D // 128   # 4 contraction chunks
    NB = B // 128   # 8 output row tiles
    CLIP_HI = (1.0 - EPS) * S
    CLIP_LO = -CLIP_HI

    with tile.TileContext(nc) as tc:
        with (
            tc.tile_pool(name="const", bufs=1) as constp,
            tc.tile_pool(name="embp", bufs=ND) as embp,
            tc.tile_pool(name="kp", bufs=12) as kp,
            tc.tile_pool(name="knp", bufs=16) as knp,
            tc.tile_pool(name="sqp", bufs=6) as sqp,
            tc.tile_pool(name="tp", bufs=2) as tp,
            tc.tile_pool(name="invp", bufs=2) as invp,
            tc.tile_pool(name="outp", bufs=16) as outp,
            tc.tile_pool(name="fxp", bufs=2) as fxp,
            tc.tile_pool(name="smp", bufs=2) as smp,
            tc.tile_pool(name="psn", bufs=2, space="PSUM") as psn,
            tc.tile_pool(name="psm", bufs=6, space="PSUM") as psm,
        ):
            ones_f = constp.tile([128, 128], F32, name="ones_f", tag="ones_f")
            nc.vector.memset(ones_f[:], 1.0)
            ones = constp.tile([128, 128], F32R, name="ones", tag="ones")
            nc.vector.tensor_copy(ones[:], ones_f[:])
            nhpi = constp.tile([128, 1], F32, name="nhpi", tag="nhpi")
            nc.vector.memset(nhpi[:], -math.pi / 2)

            ets = []
            for d in range(ND):
                et = embp.tile([128, B], F32, name=f"et{d}", tag="et")
                nc.sync.dma_start(et[:], embT[d * 128:(d + 1) * 128, :])
                etr = embp.tile([128, B], F32R, name=f"etr{d}", tag="etr")
                nc.vector.tensor_copy(etr[:], et[:])
                ets.append(etr)

            for ci, (c0, cn) in enumerate(_c_tiles()):
                kts = []
                for d in range(ND):
                    kt = kp.tile([128, cn], F32, name=f"k_{ci}_{d}", tag="k",
                                 padded_shape=[128, 512])
                    nc.sync.dma_start(
                        kt[:], ksh[d * 128:(d + 1) * 128, c0:c0 + cn])
                    kts.append(kt)

                # column norm^2, broadcast to all 128 partitions via ones-matmul
                nsps = psn.tile([128, cn], F32, name=f"ns_{ci}", tag="ns",
                                padded_shape=[128, 512])
                for d in range(ND):
                    sq = sqp.tile([128, cn], F32R, name=f"sq_{ci}_{d}",
                                  tag="sq", padded_shape=[128, 512])
                    nc.scalar.square(sq[:], kts[d][:])
                    nc.tensor.matmul(nsps[:], ones[:], sq[:],
                                     start=(d == 0), stop=(d == ND - 1))

                # inv64 = S / clip(sqrt(ns), 1e-5)
                tt = tp.tile([128, cn], F32, name=f"t_{ci}", tag="t",
                             padded_shape=[128, 512])
                nc.scalar.activation(tt[:], nsps[:], AF.Sqrt, 0.0,
                                     1.0 / (S * S))
                nc.vector.tensor_scalar_max(tt[:], tt[:], 1e-5 / S)
                inv = invp.tile([128, cn], F32, name=f"inv_{ci}", tag="inv",
                                padded_shape=[128, 512])
                nc.vector.reciprocal(inv[:], tt[:])
                kns = []
                for d in range(ND):
                    kn = knp.tile([128, cn], F32R, name=f"kn_{ci}_{d}",
                                  tag="kn", padded_shape=[128, 512])
                    nc.vector.tensor_mul(kn[:], kts[d][:], inv[:])
                    kns.append(kn)

                # main matmuls: psum[b_tile, c_tile] = emb @ kn (f32r full rate)
                for b in range(NB):
                    ps = psm.tile([128, cn], F32, name=f"ps_{ci}_{b}",
                                  tag="ps", padded_shape=[128, 512])
                    for d in range(ND):
                        nc.tensor.matmul(
                            ps[:],
                            ets[d][:, b * 128:(b + 1) * 128],
                            kns[d][:],
                            start=(d == 0), stop=(d == ND - 1))
                    ot = outp.tile([128, cn], F32, name=f"o_{ci}_{b}",
                                   tag="o", padded_shape=[128, 512])
                    nc.vector.tensor_scalar(ot[:], ps[:], CLIP_HI, CLIP_LO,
                                            ALU.min, ALU.max)
                    nc.sync.dma_start(
                        out[b * 128:(b + 1) * 128, c0:c0 + cn], ot[:])

            # ---- label fix-up values (per batch row) ----
            for r in range(NB):
                rs = slice(r * 128, (r + 1) * 128)
                er = fxp.tile([128, D], F32, name=f"er{r}", tag="er")
                nc.sync.dma_start(er[:], emb[rs, :])
                kl = fxp.tile([128, D], F32, name=f"kl{r}", tag="kl")
                nc.sync.dma_start(kl[:], klabT[rs, :])
                nr = smp.tile([128, 1], F32, name=f"nr{r}", tag="nr")
                nc.sync.dma_start(nr[:], norms[rs, :])

                tmp0 = fxp.tile([128, D], F32, name=f"tmp0_{r}", tag="tmp0")
                nc.vector.tensor_mul(tmp0[:], er[:], kl[:])
                dot = smp.tile([128, 1], F32, name=f"dot{r}", tag="dot")
                nc.vector.tensor_reduce(dot[:], tmp0[:],
                                        axis=mybir.AxisListType.X, op=ALU.add)
                tmp1 = fxp.tile([128, D], F32, name=f"tmp1_{r}", tag="tmp1")
                nc.vector.tensor_mul(tmp1[:], kl[:], kl[:])
                nsq = smp.tile([128, 1], F32, name=f"nsq{r}", tag="nsq")
                nc.vector.tensor_reduce(nsq[:], tmp1[:],
                                        axis=mybir.AxisListType.X, op=ALU.add)

                st = smp.tile([128, 1], F32, name=f"st{r}", tag="st")
                nc.scalar.sqrt(st[:], nsq[:])
                nc.vector.tensor_scalar_max(st[:], st[:], 1e-5)
                iv = smp.tile([128, 1], F32, name=f"iv{r}", tag="iv")
                nc.vector.reciprocal(iv[:], st[:])
                x = smp.tile([128, 1], F32, name=f"x{r}", tag="x")
                nc.vector.tensor_mul(x[:], dot[:], iv[:])
                nc.vector.tensor_scalar(x[:], x[:], 1.0 - EPS, -(1.0 - EPS),
                                        ALU.min, ALU.max)

                # ms = clip(norms, 1e-3, 100) * H / (100 + eps)  (always in (0, 1))
                ms = smp.tile([128, 1], F32, name=f"ms{r}", tag="ms")
                nc.vector.tensor_scalar(ms[:], nr[:], 1e-3, 100.0,
                                        ALU.max, ALU.min)
                nc.vector.tensor_scalar_mul(ms[:], ms[:], H / (BSTD + EPS))

                # theta = pi/2 - arctan(x / sqrt(1 - x^2)) + M*ms, clipped
                x2 = smp.tile([128, 1], F32, name=f"x2{r}", tag="x2")
                nc.scalar.square(x2[:], x[:])
                w = smp.tile([128, 1], F32, name=f"w{r}", tag="w")
                nc.scalar.activation(w[:], x2[:], AF.Sqrt, 1.0, -1.0)
                wi = smp.tile([128, 1], F32, name=f"wi{r}", tag="wi")
                nc.vector.reciprocal(wi[:], w[:])
                q = smp.tile([128, 1], F32, name=f"q{r}", tag="q")
                nc.vector.tensor_mul(q[:], x[:], wi[:])
                at = smp.tile([128, 1], F32, name=f"at{r}", tag="at")
                nc.scalar.activation(at[:], q[:], AF.Arctan)
                msb = smp.tile([128, 1], F32, name=f"msb{r}", tag="msb")
                nc.vector.tensor_scalar(msb[:], ms[:], M_MARGIN, math.pi / 2,
                                        ALU.mult, ALU.add)
                th = smp.tile([128, 1], F32, name=f"th{r}", tag="th")
                nc.vector.tensor_sub(th[:], msb[:], at[:])
                nc.vector.tensor_scalar(th[:], th[:], math.pi - EPS, EPS,
                                        ALU.min, ALU.max)

                # sin(theta - pi/2) = -cos(theta)
                sn = smp.tile([128, 1], F32, name=f"sn{r}", tag="sn")
                nc.scalar.activation(sn[:], th[:], AF.Sin, nhpi[:])
                # val = (cos(theta) - (HEAD_B - M*ms)) * S = -S*sn - S*HEAD_B + S*M*ms
                v1 = smp.tile([128, 1], F32, name=f"v1{r}", tag="v1")
                nc.vector.tensor_scalar(v1[:], ms[:], S * M_MARGIN,
                                        -S * HEAD_B, ALU.mult, ALU.add)
                v2 = smp.tile([128, 1], F32, name=f"v2{r}", tag="v2")
                nc.vector.tensor_scalar_mul(v2[:], sn[:], -S)
                fv = smp.tile([128, 1], F32, name=f"fv{r}", tag="fv")
                nc.vector.tensor_add(fv[:], v1[:], v2[:])
                nc.sync.dma_start(fixv[rs, :], fv[:])

    nc.compile()
    return nc


def _get_nc():
    if "nc" not in _nc_cache:
        _nc_cache["nc"] = build_nc()
    return _nc_cache["nc"]


def kernel(embbedings, norms, kernel, label):
    emb = np.ascontiguousarray(np.asarray(embbedings, dtype=np.float32))
    kfull = np.asarray(kernel, dtype=np.float32)
    nrm = np.ascontiguousarray(np.asarray(norms, dtype=np.float32).reshape(B, 1))
    lab = np.asarray(label).astype(np.int64)

    kpad = np.zeros((D, CPAD), dtype=np.float32)
    kpad[:, :C] = kfull
    embT = np.ascontiguousarray(emb.T)
    klabT = np.ascontiguousarray(kfull[:, lab].T)

    in_maps = []
    for j in range(NCORES):
        in_maps.append({
            "ksh": np.ascontiguousarray(kpad[:, j * CS:(j + 1) * CS]),
            "embT": embT,
            "emb": emb,
            "klabT": klabT,
            "norms": nrm,
        })

    nc = _get_nc()
    res = run_bass_kernel_spmd(nc, in_maps, core_ids=list(range(NCORES)))
    results = res.results

    full = np.concatenate([results[j]["out"] for j in range(NCORES)], axis=1)
    outv = full[:, :C]
    outv[np.arange(B), lab] = results[0]["fixv"][:, 0]
    return outv
